# revision 14
# baseline (speedup 1.0000x reference)
"""AAM-Softmax (ArcFace) loss + top-1 accuracy on 8 TRN2 NeuronCores.

Math (reference): cosine = l2n(x) @ l2n(w).T ; the angular margin (phi) only
affects the label column; loss = mean CE of SCALE*logits;
prec1 = 100*mean(argmax==label).

v3 structure:
- Host: x and w l2-normalized in fp32, quantized to fp8; cos_label recomputed
  on host from the same fp8 values the device sees; phi15/tau/elab shipped as
  f32 scalars per batch row. No device prologue.
- Classes: 6144/core sharded (3 blocks x 2048) + the last 848 classes
  REPLICATED on every core (block R, computed last). The replicated tail
  removes the final AllReduce from the critical path: AR1 (blocks 0+1) hides
  behind block 2+R, AR2 (block 2 only) hides behind block R's compute.
- fp8 DoubleRow matmuls (k-pair interleaved rhs), c-outer so 4 matmuls share
  each LDWEIGHTS; psum bufs=2 x [128,2048] f32 fills all 8 PSUM banks.
- Per unit (m, block): ScalarE exp(15*cos) accum_out -> sumexp partials
  (the pipeline pacer at ~2.1us/unit), VectorE compare+reduce -> counts.
- Startup DMAs split across sync (w) and gpsimd (x, scal) queues.
"""

import math
import sys

import numpy as np

if "/opt/trn_rl_repo" not in sys.path:
    sys.path.insert(0, "/opt/trn_rl_repo")

import ml_dtypes

N_CORES = 8
B, D, C = 2048, 512, 50000
MT = B // 128               # 16 batch tiles
CPS = 6144                  # sharded classes per core (3 x 2048)
NBLK = 3
RW = C - N_CORES * CPS      # replicated tail classes: 848
RPAD = 1024

MARGIN = 0.3
SCALE = 15.0
COS_M = math.cos(MARGIN)
SIN_M = math.sin(MARGIN)
TH = math.cos(math.pi - MARGIN)
MM = math.sin(math.pi - MARGIN) * MARGIN

_CACHE = {}


def _patch_act_tables():
    """Make natural_log_exp_and_others the only set offering Exp/Ln/Square,
    so bacc's table-load pass never ping-pongs between sets."""
    import concourse.bacc as bacc_mod
    import concourse.hw_specs as hw_specs
    from concourse import mybir

    if getattr(bacc_mod, "_aam_table_patch", False):
        return
    AF = mybir.ActivationFunctionType
    orig = hw_specs.get_activation_tables
    steal = {AF.Exp, AF.Ln, AF.Square, AF.Sign}
    target = "natural_log_exp_and_others"

    def patched(arch):
        t = orig(arch)
        return {
            name: (fns if name == target else fns - steal)
            for name, fns in t.items()
        }

    bacc_mod.get_activation_tables = patched
    bacc_mod._aam_table_patch = True


def _build():
    from concourse import bacc, mybir
    import concourse.tile as tile

    _patch_act_tables()

    f32 = mybir.dt.float32
    bf = mybir.dt.bfloat16
    f8 = mybir.dt.float8e4
    AF = mybir.ActivationFunctionType
    OP = mybir.AluOpType
    AX = mybir.AxisListType.X
    RG = [list(range(N_CORES))]
    DR = mybir.MatmulPerfMode.DoubleRow

    nc = bacc.Bacc("TRN2", target_bir_lowering=False, debug=False,
                   enable_asserts=False, num_devices=N_CORES)

    # xbT: [p, c*B + b] = x_norm fp8, K-major (c in 0..3 picks k-chunk)
    xbt_d = nc.dram_tensor("xbT", [128, 4 * B], f8, kind="ExternalInput").ap()
    # wsT: [p, c*2*CPS + j*2 + i] = shard w_norm fp8, k-pair interleaved
    wst_d = nc.dram_tensor("wsT", [128, 2 * 2 * CPS], f8,
                           kind="ExternalInput").ap()
    # wrT: replicated tail classes, same layout
    wrt_d = nc.dram_tensor("wrT", [128, 2 * 2 * RPAD], f8,
                           kind="ExternalInput").ap()
    # scal: cols 0:16 phi15, 16:32 tau, 32:48 elab  (per batch row, [p, m])
    sc_d = nc.dram_tensor("scal", [128, 3 * MT], f32, kind="ExternalInput").ap()
    out_d = nc.dram_tensor("out", [1, 2], f32, kind="ExternalOutput").ap()

    with tile.TileContext(nc) as tc:
        with tc.tile_pool(name="persist", bufs=1) as per, \
             tc.tile_pool(name="wt", bufs=2) as wpool, \
             tc.tile_pool(name="ex", bufs=4) as expool, \
             tc.tile_pool(name="scr", bufs=3) as scr, \
             tc.tile_pool(name="psum", bufs=2, space="PSUM") as psum, \
             tc.tile_pool(name="dram", bufs=2, space="DRAM") as dram:

            xT = per.tile([128, 4, B], f8, tag="xT")
            # x on the scalar hwdge queue (2 chunks), w on sync: parallel
            for h in range(2):
                nc.scalar.dma_start(
                    out=xT[:, 2 * h:2 * h + 2, :],
                    in_=xbt_d[:, h * 2 * B:(h + 1) * 2 * B].rearrange(
                        "p (c b) -> p c b", c=2))
            scal = per.tile([128, 3 * MT], f32, tag="scal")
            nc.scalar.dma_start(out=scal[:], in_=sc_d[:])

            ones = per.tile([128, 1], f32, tag="ones")
            nc.vector.memset(ones[:], 1.0)

            # per-block accumulators (separate tiles: no WAW chains)
            sacc = [per.tile([128, MT], f32, tag=f"s{b}", name=f"sacc{b}")
                    for b in range(4)]
            cacc = [per.tile([128, MT], f32, tag=f"c{b}", name=f"cacc{b}")
                    for b in range(4)]
            arinA = per.tile([128, 32], bf, tag="arinA")
            arinBa = per.tile([128, 16], bf, tag="arinBa")
            arinBb = per.tile([128, 16], bf, tag="arinBb")

            w_tiles = {}

            def w_load(b, nchunk=1):
                wt = wpool.tile([128, 2, 2048, 2], f8, tag="wT")
                w_tiles[b] = wt
                bs = b * 2048
                for c in range(2):
                    off = c * 2 * CPS + bs * 2
                    for h in range(nchunk):
                        w = 2048 // nchunk
                        nc.sync.dma_start(
                            out=wt[:, c, h * w:(h + 1) * w, :],
                            in_=wst_d[:, off + h * w * 2:off + (h + 1) * w * 2]
                                .rearrange("p (j i) -> p j i", i=2))

            w_load(0, nchunk=4)

            wr = per.tile([128, 2, RPAD, 2], f8, tag="wr")

            def wr_load():
                for c in range(2):
                    nc.sync.dma_start(
                        out=wr[:, c, :, :],
                        in_=wrt_d[:, c * 2 * RPAD:(c + 1) * 2 * RPAD]
                            .rearrange("p (j i) -> p j i", i=2))

            def unit(m, wt, gw, nw, sac, cac):
                ms = slice(m * 128, (m + 1) * 128)
                ps = psum.tile([128, 2048], f32, tag="ps")
                for c in range(2):
                    for s in range(max(1, gw // 512)):
                        cw = min(512, gw - s * 512)
                        nc.tensor.matmul(
                            ps[:, s * 512:s * 512 + cw],
                            lhsT=xT[:, 2 * c:2 * c + 2, ms],
                            rhs=wt[:, c, s * 512:s * 512 + cw, :]
                                .rearrange("p n i -> p i n"),
                            start=(c == 0), stop=(c == 1),
                            perf_mode=DR)
                ex = expool.tile([128, 2048], bf, tag="ex")
                nc.scalar.activation(ex[:, :nw], ps[:, :nw], AF.Exp,
                                     scale=SCALE, accum_out=sac[:, m:m + 1])
                cn = scr.tile([128, 2048], bf, tag="cn")
                nc.vector.tensor_scalar(
                    out=cn[:, :nw], in0=ex[:, :nw],
                    scalar1=scal[:, 16 + m:17 + m], scalar2=None,
                    op0=OP.is_gt, op1=OP.add, accum_out=cac[:, m:m + 1])

            for b in range(NBLK):
                for m in range(MT):
                    if b + 1 < NBLK and m == 1:
                        w_load(b + 1)
                    if b == NBLK - 1 and m == 1:
                        wr_load()
                    unit(m, w_tiles[b], 2048, 2048, sacc[b], cacc[b])
                    if b == 2 and m == 7:
                        # AllReduce #2a: block 2, rows m 0-7 (early half)
                        nc.vector.tensor_copy(arinBa[:, 0:8],
                                              sacc[2][:, 0:8])
                        nc.vector.tensor_copy(arinBa[:, 8:16],
                                              cacc[2][:, 0:8])
                        cin2a = dram.tile([128, 16], bf)
                        cout2a = dram.tile([128, 16], bf,
                                           addr_space="Shared")
                        nc.gpsimd.dma_start(out=cin2a[:], in_=arinBa[:])
                        nc.gpsimd.collective_compute(
                            "AllReduce", OP.add, replica_groups=RG,
                            ins=[cin2a[:]], outs=[cout2a[:]])

                if b == 1:
                    # AllReduce #1 (bf16): blocks 0-1, hidden behind 2 + R
                    nc.vector.tensor_tensor(out=arinA[:, 0:16],
                                            in0=sacc[0][:], in1=sacc[1][:],
                                            op=OP.add)
                    nc.vector.tensor_tensor(out=arinA[:, 16:32],
                                            in0=cacc[0][:], in1=cacc[1][:],
                                            op=OP.add)
                    cin1 = dram.tile([128, 32], bf)
                    cout1 = dram.tile([128, 32], bf, addr_space="Shared")
                    nc.gpsimd.dma_start(out=cin1[:], in_=arinA[:])
                    nc.gpsimd.collective_compute(
                        "AllReduce", OP.add, replica_groups=RG,
                        ins=[cin1[:]], outs=[cout1[:]])

            # AllReduce #2b: block 2, rows m 8-15; hidden behind block R
            nc.vector.tensor_copy(arinBb[:, 0:8], sacc[2][:, 8:16])
            nc.vector.tensor_copy(arinBb[:, 8:16], cacc[2][:, 8:16])
            cin2b = dram.tile([128, 16], bf)
            cout2b = dram.tile([128, 16], bf, addr_space="Shared")
            nc.gpsimd.dma_start(out=cin2b[:], in_=arinBb[:])
            nc.gpsimd.collective_compute(
                "AllReduce", OP.add, replica_groups=RG,
                ins=[cin2b[:]], outs=[cout2b[:]])

            # replicated tail block (every core computes the same 848 classes)
            for m in range(MT):
                unit(m, wr, RPAD, RW, sacc[3], cacc[3])

            tot1 = per.tile([128, 32], bf, tag="tot1")
            nc.sync.dma_start(out=tot1[:], in_=cout1[:])
            tot2 = per.tile([128, 32], bf, tag="tot2")
            nc.sync.dma_start(out=tot2[:, 0:8], in_=cout2a[:, 0:8])
            nc.sync.dma_start(out=tot2[:, 8:16], in_=cout2b[:, 0:8])
            nc.sync.dma_start(out=tot2[:, 16:24], in_=cout2a[:, 8:16])
            nc.sync.dma_start(out=tot2[:, 24:32], in_=cout2b[:, 8:16])
            tot = per.tile([128, 32], f32, tag="tot")
            nc.vector.tensor_tensor(out=tot[:], in0=tot1[:], in1=tot2[:],
                                    op=OP.add)
            totS = per.tile([128, MT], f32, tag="totS")
            nc.vector.tensor_tensor(out=totS[:], in0=tot[:, 0:16],
                                    in1=sacc[3][:], op=OP.add)
            totC = per.tile([128, MT], f32, tag="totC")
            nc.vector.tensor_tensor(out=totC[:], in0=tot[:, 16:32],
                                    in1=cacc[3][:], op=OP.add)

            # loss: ln(sumexp - elab + tau) - phi15 ; prec: count == 1
            sp1 = per.tile([128, MT], f32, tag="sp1")
            nc.vector.tensor_tensor(out=sp1[:], in0=totS[:],
                                    in1=scal[:, 32:48], op=OP.subtract)
            sp2 = per.tile([128, MT], f32, tag="sp2")
            nc.vector.tensor_tensor(out=sp2[:], in0=sp1[:],
                                    in1=scal[:, 16:32], op=OP.add)
            lnS = per.tile([128, MT], f32, tag="lnS")
            nc.scalar.activation(lnS[:], sp2[:], AF.Ln)
            nll = per.tile([128, MT], f32, tag="nll")
            nc.vector.tensor_tensor(out=nll[:], in0=lnS[:],
                                    in1=scal[:, 0:16], op=OP.subtract)
            pack = per.tile([128, 2], f32, tag="pack")
            nc.vector.reduce_sum(out=pack[:, 0:1], in_=nll[:], axis=AX)
            corr = per.tile([128, MT], f32, tag="corr")
            nc.vector.tensor_scalar(out=corr[:], in0=totC[:],
                                    scalar1=1.0, scalar2=None,
                                    op0=OP.is_equal)
            nc.vector.reduce_sum(out=pack[:, 1:2], in_=corr[:], axis=AX)
            fin = psum.tile([128, 2048], f32, tag="ps")
            nc.tensor.matmul(fin[:1, :2], lhsT=ones[:], rhs=pack[:],
                             start=True, stop=True)
            osb = per.tile([1, 2], f32, tag="osb")
            nc.scalar.mul(osb[:, 0:1], fin[:1, 0:1], 1.0 / B)
            nc.scalar.mul(osb[:, 1:2], fin[:1, 1:2], 100.0 / B)
            nc.sync.dma_start(out=out_d[:], in_=osb[:])

    nc.compile()
    return nc


def _get_nc():
    if "nc" not in _CACHE:
        _CACHE["nc"] = _build()
    return _CACHE["nc"]


def _wlay(shard):
    """[J, 512] fp8 -> [128, 2*2*J] interleaved k-pair layout."""
    J = shard.shape[0]
    t = shard.T.reshape(2, 2, 128, J)                 # [c, i, p, j]
    return np.ascontiguousarray(
        t.transpose(2, 0, 3, 1).reshape(128, 2 * 2 * J))


def kernel(x: np.ndarray, weight: np.ndarray, label: np.ndarray, **_ignored):
    from concourse.bass_utils import run_bass_kernel_spmd

    f8 = ml_dtypes.float8_e4m3
    x = np.asarray(x, dtype=np.float32)
    weight = np.asarray(weight, dtype=np.float32)
    lab = np.asarray(label).astype(np.int64)

    # host: exact l2 normalize, then fp8 quantize
    xn = x / np.maximum(np.sqrt((x * x).sum(1, keepdims=True)), 1e-12)
    wn = weight / np.maximum(np.sqrt((weight * weight).sum(1, keepdims=True)),
                             1e-12)
    xq = xn.astype(f8)
    wq = wn.astype(f8)

    # label-column math in fp64 from the same fp8 values the device sees
    xqf = xq.astype(np.float64)
    wqf = wq[lab].astype(np.float64)
    cosl = (xqf * wqf).sum(1)
    sinl = np.sqrt(np.clip(1.0 - cosl * cosl, 0.0, 1.0))
    phi = cosl * COS_M - sinl * SIN_M
    phi = np.where(cosl - TH > 0, phi, cosl - MM)
    phi15 = (SCALE * phi).astype(np.float32)
    tau = np.exp(SCALE * phi).astype(np.float32)
    elab = np.exp(SCALE * cosl).astype(np.float32)

    # [p, m] layout: batch row i = m*128 + p
    def pm(v):
        return np.ascontiguousarray(v.reshape(MT, 128).T.astype(np.float32))

    scal = np.ascontiguousarray(
        np.concatenate([pm(phi15), pm(tau), pm(elab)], axis=1))

    # xbT: [p, c*B + b] with element (p, c, i) = xq[i, c*128 + p]
    xbT = np.ascontiguousarray(
        xq.T.reshape(4, 128, B).transpose(1, 0, 2).reshape(128, 4 * B))

    tail = np.zeros((RPAD, D), dtype=f8)
    tail[:RW] = wq[N_CORES * CPS:]
    wrT = _wlay(tail)

    in_maps = []
    for k in range(N_CORES):
        wsT = _wlay(wq[k * CPS:(k + 1) * CPS])
        in_maps.append({"xbT": xbT, "wsT": wsT, "wrT": wrT, "scal": scal})

    nc = _get_nc()
    res = run_bass_kernel_spmd(nc, in_maps, core_ids=list(range(N_CORES)))
    out = res.results[0]["out"]
    loss = np.float32(out[0, 0])
    prec1 = np.float32(out[0, 1])
    return (loss, prec1)


if __name__ == "__main__":
    pass
